# revision 4
# baseline (speedup 1.0000x reference)
"""DeepSpeed MoE block on 8 Trainium2 NeuronCores, expert-parallel.

Host: LayerNorm + gate + capacity-aware top-2 routing (cheap, O(T*E)) and the
dispatch/combine gathers. Device (per core = one expert): the dense expert MLP
y = gelu(xbuf @ W1 + b1) @ W2 + b2 over the expert's [C, H] token buffer —
>99.9% of the FLOPs — in bf16 with fp32 PSUM accumulation.

Self-contained: hardcodes shapes B=2, S=2048, H=1024, E=8, F=4096, K=2, C=1024.
"""

import numpy as np
import ml_dtypes

import concourse.bacc as bacc
import concourse.mybir as mybir
from concourse import tile
from concourse.bass_utils import run_bass_kernel_spmd

LN_EPS = 1e-5
GATE_EPS = 1e-9
TOP_K = 2
CAP_FACTOR = 1.0

N_CORES = 8
P = 128
H, F = 1024, 4096
E = 8
C = 1024  # int(ceil(TOP_K * 4096 / 8 * 1.0))

KT1 = H // P   # 8  k-tiles, MLP1 contraction
MT1 = F // P   # 32 m-tiles, MLP1 output partitions (F)
KT2 = F // P   # 32 k-tiles, MLP2 contraction
MT2 = H // P   # 8  m-tiles, MLP2 output partitions (H)
NCH = 512      # moving free dim per matmul
NN = C // NCH  # 2 column chunks

BF16 = mybir.dt.bfloat16
F32 = mybir.dt.float32
np_bf16 = np.dtype(ml_dtypes.bfloat16)


def build_expert_program(repeats: int = 1):
    """One expert's MLP: yT[m,p,c] = (gelu(x @ W1 + b1) @ W2 + b2)^T.

    Inputs are pre-laid-out host-side so every DMA is contiguous per partition:
      xt  [P, KT1, C]       bf16   xt[p,k,c]    = xbuf[c, k*P+p]
      w1  [P, MT1, KT1, P]  bf16   w1[p,m,k,q]  = W1[k*P+p, m*P+q]
      w2  [P, MT2, KT2, P]  bf16   w2[p,m,k,q]  = W2[k*P+p, m*P+q]
      b1  [P, MT1]          f32    b1[p,m]      = b1[m*P+p]
      b2  [P, MT2]          f32
    Output:
      yt  [MT2, P, C]       f32    yt[m,p,c]    = y[c, m*P+p]
    """
    nc = bacc.Bacc("TRN2", target_bir_lowering=False, debug=False,
                   num_devices=N_CORES)
    xt = nc.dram_tensor("xt", [P, KT1, C], BF16, kind="ExternalInput").ap()
    w1 = nc.dram_tensor("w1", [P, MT1, KT1, P], BF16, kind="ExternalInput").ap()
    w2 = nc.dram_tensor("w2", [P, MT2, KT2, P], BF16, kind="ExternalInput").ap()
    b1 = nc.dram_tensor("b1", [P, MT1], F32, kind="ExternalInput").ap()
    b2 = nc.dram_tensor("b2", [P, MT2], F32, kind="ExternalInput").ap()
    yt = nc.dram_tensor("yt", [MT2, P, C], F32, kind="ExternalOutput").ap()

    with tile.TileContext(nc) as tc:
        with (
            tc.tile_pool(name="const", bufs=1) as const,
            tc.tile_pool(name="h1", bufs=1) as h1p,
            tc.tile_pool(name="w1p", bufs=3) as w1p,
            tc.tile_pool(name="w2p", bufs=3) as w2p,
            tc.tile_pool(name="psum", bufs=4, space="PSUM") as psp,
            tc.tile_pool(name="out", bufs=3) as outp,
        ):
            xt_sb = const.tile([P, KT1, C], BF16)
            for k in range(KT1):
                nc.sync.dma_start(xt_sb[:, k], xt[:, k])
            b1_sb = const.tile([P, MT1], F32)
            nc.sync.dma_start(b1_sb[:], b1[:])
            b2_sb = const.tile([P, MT2], F32)
            nc.sync.dma_start(b2_sb[:], b2[:])

            h1_sb = [
                h1p.tile([P, C], BF16, tag=f"h1_{k}", name=f"h1_{k}")
                for k in range(KT2)
            ]

            for _ in range(repeats):
                # MLP1: h1^T[F, C] = gelu(W1^T x^T + b1)
                for m in range(MT1):
                    w1t = w1p.tile([P, KT1, P], BF16)
                    nc.sync.dma_start(w1t[:], w1[:, m])
                    ps = [psp.tile([P, NCH], F32, name=f"ps{n}", tag="ps") for n in range(NN)]
                    for k in range(KT1):
                        for n in range(NN):
                            nc.tensor.matmul(
                                ps[n][:], w1t[:, k], xt_sb[:, k, n * NCH:(n + 1) * NCH],
                                start=(k == 0), stop=(k == KT1 - 1),
                            )
                    for n in range(NN):
                        nc.scalar.activation(
                            h1_sb[m][:, n * NCH:(n + 1) * NCH], ps[n][:],
                            mybir.ActivationFunctionType.Gelu,
                            bias=b1_sb[:, m:m + 1],
                        )
                # MLP2: y^T[H, C] = W2^T h1^T + b2
                for m in range(MT2):
                    w2t = w2p.tile([P, KT2, P], BF16)
                    nc.sync.dma_start(w2t[:], w2[:, m])
                    ps = [psp.tile([P, NCH], F32, name=f"ps{n}", tag="ps") for n in range(NN)]
                    for k in range(KT2):
                        for n in range(NN):
                            nc.tensor.matmul(
                                ps[n][:], w2t[:, k], h1_sb[k][:, n * NCH:(n + 1) * NCH],
                                start=(k == 0), stop=(k == KT2 - 1),
                            )
                    o_sb = outp.tile([P, C], F32)
                    for n in range(NN):
                        nc.vector.tensor_scalar_add(
                            o_sb[:, n * NCH:(n + 1) * NCH], ps[n][:], b2_sb[:, m:m + 1]
                        )
                    nc.sync.dma_start(yt[m], o_sb[:])

    nc.compile()
    return nc


_PROGRAM_CACHE = {}


def _get_program(repeats: int = 1):
    if repeats not in _PROGRAM_CACHE:
        _PROGRAM_CACHE[repeats] = build_expert_program(repeats)
    return _PROGRAM_CACHE[repeats]


def host_routing(x, gamma, beta, wg):
    """LayerNorm + softmax gate + capacity-aware top-2 routing (numpy fp32)."""
    B, S, _ = x.shape
    T = B * S
    K = TOP_K

    xf = x.reshape(T, H).astype(np.float32)
    mu = xf.mean(axis=-1, keepdims=True, dtype=np.float32)
    var = ((xf - mu) ** 2).mean(axis=-1, keepdims=True, dtype=np.float32)
    xn = (xf - mu) * (1.0 / np.sqrt(var + LN_EPS)) * gamma + beta

    logits = xn @ wg
    mx = logits.max(axis=-1, keepdims=True)
    eg = np.exp(logits - mx)
    gates = eg / eg.sum(axis=-1, keepdims=True)

    ordr = np.argsort(-gates, axis=1, kind="stable")[:, :K]
    idx = ordr.astype(np.int64)
    vals = np.take_along_axis(gates, ordr, axis=1)

    counts = np.zeros((E,), np.int64)
    pos_k = np.zeros((T, K), np.int64)
    keep_k = np.zeros((T, K), bool)
    for j in range(K):
        oh = np.zeros((T, E), np.int64)
        oh[np.arange(T), idx[:, j]] = 1
        loc = np.cumsum(oh, axis=0) - 1 + counts[None, :]
        pos_k[:, j] = (loc * oh).sum(axis=1)
        counts = counts + oh.sum(axis=0)
        keep_k[:, j] = pos_k[:, j] < C

    kept = vals * keep_k
    w = kept / (kept.sum(axis=1, keepdims=True) + GATE_EPS)

    me = gates.mean(axis=0)
    ce = np.zeros((E,), np.float32)
    np.add.at(ce, idx[:, 0], 1.0)
    ce /= T
    l_aux = np.float32((me * ce).mean() * E * E)

    tok_for_slot = np.zeros((E, C), np.int64)
    for j in range(K):
        mk = keep_k[:, j]
        tok_for_slot[idx[mk, j], pos_k[mk, j]] = np.arange(T)[mk]

    return xn, idx, pos_k, keep_k, w, l_aux, counts.astype(np.float32), tok_for_slot


def _prep_in_maps(x, gamma, beta, wg, W1, b1, W2, b2):
    xn, idx, pos_k, keep_k, w, l_aux, counts, tok_for_slot = host_routing(
        np.asarray(x), np.asarray(gamma, np.float32), np.asarray(beta, np.float32),
        np.asarray(wg, np.float32))

    W1 = np.asarray(W1, np.float32)
    W2 = np.asarray(W2, np.float32)
    b1 = np.asarray(b1, np.float32)
    b2 = np.asarray(b2, np.float32)

    in_maps = []
    for e in range(E):
        xbuf = xn[tok_for_slot[e]]                       # [C, H]
        xt = np.ascontiguousarray(
            xbuf.T.reshape(KT1, P, C).transpose(1, 0, 2)).astype(np_bf16)
        w1r = np.ascontiguousarray(
            W1[e].reshape(KT1, P, MT1, P).transpose(1, 2, 0, 3)).astype(np_bf16)
        w2r = np.ascontiguousarray(
            W2[e].reshape(KT2, P, MT2, P).transpose(1, 2, 0, 3)).astype(np_bf16)
        b1r = np.ascontiguousarray(b1[e].reshape(MT1, P).T)
        b2r = np.ascontiguousarray(b2[e].reshape(MT2, P).T)
        in_maps.append({"xt": xt, "w1": w1r, "w2": w2r, "b1": b1r, "b2": b2r})

    routing = (xn, idx, pos_k, keep_k, w, l_aux, counts, tok_for_slot)
    return in_maps, routing


def _combine(x, results, routing):
    xn, idx, pos_k, keep_k, w, l_aux, counts, tok_for_slot = routing
    T = x.shape[0] * x.shape[1]
    # y[e] in [C, H] from yt [MT2, P, C]
    y = np.stack([
        results[e]["yt"].reshape(H, C).T for e in range(E)
    ])                                                   # [E, C, H]
    outf = np.zeros((T, H), np.float32)
    for j in range(TOP_K):
        mk = keep_k[:, j]
        outf[mk] += w[mk, j, None] * y[idx[mk, j], pos_k[mk, j]]
    out = np.asarray(x, np.float32) + outf.reshape(x.shape)
    return out, np.float32(l_aux), counts


def kernel(x, gamma, beta, wg, W1, b1, W2, b2):
    nc = _get_program(1)
    in_maps, routing = _prep_in_maps(x, gamma, beta, wg, W1, b1, W2, b2)
    res = run_bass_kernel_spmd(nc, in_maps, list(range(N_CORES)))
    return _combine(np.asarray(x), res.results, routing)


# revision 5
# speedup vs baseline: 899.8403x; 899.8403x over previous
"""DeepSpeed MoE block on 8 Trainium2 NeuronCores, expert-parallel.

Host: LayerNorm + gate + capacity-aware top-2 routing (cheap, O(T*E)) and the
dispatch/combine gathers. Device (per core = one expert): the dense expert MLP
y = gelu(xbuf @ W1 + b1) @ W2 + b2 over the expert's [C, H] token buffer —
>99.9% of the FLOPs — in bf16 with fp32 PSUM accumulation.

Self-contained: hardcodes shapes B=2, S=2048, H=1024, E=8, F=4096, K=2, C=1024.
"""

import numpy as np
import ml_dtypes

import concourse.bacc as bacc
import concourse.mybir as mybir
from concourse import tile
from concourse.bass_utils import run_bass_kernel_spmd

LN_EPS = 1e-5
GATE_EPS = 1e-9
TOP_K = 2
CAP_FACTOR = 1.0

N_CORES = 8
P = 128
H, F = 1024, 4096
E = 8
C = 1024  # int(ceil(TOP_K * 4096 / 8 * 1.0))

KT1 = H // P   # 8  k-tiles, MLP1 contraction
MT1 = F // P   # 32 m-tiles, MLP1 output partitions (F)
KT2 = F // P   # 32 k-tiles, MLP2 contraction
MT2 = H // P   # 8  m-tiles, MLP2 output partitions (H)
NCH = 512      # moving free dim per matmul
NN = C // NCH  # 2 column chunks

BF16 = mybir.dt.bfloat16
F32 = mybir.dt.float32
np_bf16 = np.dtype(ml_dtypes.bfloat16)


def _emit_body(nc, aps, sb, w1p, w2p, psp, outp):
    """One pass of the expert MLP (both matmuls + activations + output DMA)."""
    xt_sb, h1_sb, b1_sb, b2_sb = sb
    yt, w1, w2 = aps

    # MLP1: h1^T[F, C] = gelu(W1^T x^T + b1)
    for m in range(MT1):
        w1t = w1p.tile([P, KT1, P], BF16, name="w1t")
        nc.sync.dma_start(w1t[:], w1[:, m])
        ps = [psp.tile([P, NCH], F32, name=f"ps{n}", tag="ps") for n in range(NN)]
        for k in range(KT1):
            for n in range(NN):
                nc.tensor.matmul(
                    ps[n][:], w1t[:, k], xt_sb[:, k, n * NCH:(n + 1) * NCH],
                    start=(k == 0), stop=(k == KT1 - 1),
                )
        for n in range(NN):
            nc.scalar.activation(
                h1_sb[m][:, n * NCH:(n + 1) * NCH], ps[n][:],
                mybir.ActivationFunctionType.Gelu,
                bias=b1_sb[:, m:m + 1],
            )
    # MLP2: y^T[H, C] = W2^T h1^T + b2
    for m in range(MT2):
        w2t = w2p.tile([P, KT2, P], BF16, name="w2t")
        nc.sync.dma_start(w2t[:], w2[:, m])
        ps = [psp.tile([P, NCH], F32, name=f"ps{n}", tag="ps") for n in range(NN)]
        for k in range(KT2):
            for n in range(NN):
                nc.tensor.matmul(
                    ps[n][:], w2t[:, k], h1_sb[k][:, n * NCH:(n + 1) * NCH],
                    start=(k == 0), stop=(k == KT2 - 1),
                )
        o_sb = outp.tile([P, C], F32, name="o_sb")
        for n in range(NN):
            nc.vector.tensor_scalar_add(
                o_sb[:, n * NCH:(n + 1) * NCH], ps[n][:], b2_sb[:, m:m + 1]
            )
        nc.sync.dma_start(yt[m], o_sb[:])


def build_expert_program(repeats: int = 1, dyn_loop: bool = False):
    """One expert's MLP: yT[m,p,c] = (gelu(x @ W1 + b1) @ W2 + b2)^T.

    Inputs are pre-laid-out host-side so every DMA is contiguous per partition:
      xt  [P, KT1, C]       bf16   xt[p,k,c]    = xbuf[c, k*P+p]
      w1  [P, MT1, KT1, P]  bf16   w1[p,m,k,q]  = W1[k*P+p, m*P+q]
      w2  [P, MT2, KT2, P]  bf16   w2[p,m,k,q]  = W2[k*P+p, m*P+q]
      b1  [P, MT1]          f32    b1[p,m]      = b1[m*P+p]
      b2  [P, MT2]          f32
    Output:
      yt  [MT2, P, C]       f32    yt[m,p,c]    = y[c, m*P+p]

    dyn_loop=True adds a `reps` int32 [1,1] input and wraps the body in a
    runtime For_i — used for timing (same NEFF, variable iteration count).
    """
    nc = bacc.Bacc("TRN2", target_bir_lowering=False, debug=False,
                   num_devices=N_CORES)
    xt = nc.dram_tensor("xt", [P, KT1, C], BF16, kind="ExternalInput").ap()
    w1 = nc.dram_tensor("w1", [P, MT1, KT1, P], BF16, kind="ExternalInput").ap()
    w2 = nc.dram_tensor("w2", [P, MT2, KT2, P], BF16, kind="ExternalInput").ap()
    b1 = nc.dram_tensor("b1", [P, MT1], F32, kind="ExternalInput").ap()
    b2 = nc.dram_tensor("b2", [P, MT2], F32, kind="ExternalInput").ap()
    if dyn_loop:
        reps = nc.dram_tensor("reps", [1, 1], mybir.dt.int32,
                              kind="ExternalInput").ap()
    yt = nc.dram_tensor("yt", [MT2, P, C], F32, kind="ExternalOutput").ap()

    with tile.TileContext(nc) as tc:
        with (
            tc.tile_pool(name="const", bufs=1) as const,
            tc.tile_pool(name="h1", bufs=1) as h1p,
            tc.tile_pool(name="w1p", bufs=3) as w1p,
            tc.tile_pool(name="w2p", bufs=3) as w2p,
            tc.tile_pool(name="psum", bufs=4, space="PSUM") as psp,
            tc.tile_pool(name="out", bufs=3) as outp,
        ):
            xt_sb = const.tile([P, KT1, C], BF16)
            for k in range(KT1):
                nc.sync.dma_start(xt_sb[:, k], xt[:, k])
            b1_sb = const.tile([P, MT1], F32)
            nc.sync.dma_start(b1_sb[:], b1[:])
            b2_sb = const.tile([P, MT2], F32)
            nc.sync.dma_start(b2_sb[:], b2[:])

            h1_sb = [
                h1p.tile([P, C], BF16, tag=f"h1_{k}", name=f"h1_{k}")
                for k in range(KT2)
            ]

            sb = (xt_sb, h1_sb, b1_sb, b2_sb)
            aps = (yt, w1, w2)
            if dyn_loop:
                reps_sb = const.tile([1, 1], mybir.dt.int32)
                nc.sync.dma_start(reps_sb[:], reps[:])
                rv = nc.values_load(reps_sb[0:1, 0:1], min_val=0, max_val=1 << 20,
                                    skip_runtime_bounds_check=True)
                with tc.For_i(0, rv, 1):
                    _emit_body(nc, aps, sb, w1p, w2p, psp, outp)
            else:
                for _ in range(repeats):
                    _emit_body(nc, aps, sb, w1p, w2p, psp, outp)

    nc.compile()
    return nc


_PROGRAM_CACHE = {}


def _get_program(repeats: int = 1):
    if repeats not in _PROGRAM_CACHE:
        _PROGRAM_CACHE[repeats] = build_expert_program(repeats)
    return _PROGRAM_CACHE[repeats]


def host_routing(x, gamma, beta, wg):
    """LayerNorm + softmax gate + capacity-aware top-2 routing (numpy fp32)."""
    B, S, _ = x.shape
    T = B * S
    K = TOP_K

    xf = x.reshape(T, H).astype(np.float32)
    mu = xf.mean(axis=-1, keepdims=True, dtype=np.float32)
    var = ((xf - mu) ** 2).mean(axis=-1, keepdims=True, dtype=np.float32)
    xn = (xf - mu) * (1.0 / np.sqrt(var + LN_EPS)) * gamma + beta

    logits = xn @ wg
    mx = logits.max(axis=-1, keepdims=True)
    eg = np.exp(logits - mx)
    gates = eg / eg.sum(axis=-1, keepdims=True)

    ordr = np.argsort(-gates, axis=1, kind="stable")[:, :K]
    idx = ordr.astype(np.int64)
    vals = np.take_along_axis(gates, ordr, axis=1)

    counts = np.zeros((E,), np.int64)
    pos_k = np.zeros((T, K), np.int64)
    keep_k = np.zeros((T, K), bool)
    for j in range(K):
        oh = np.zeros((T, E), np.int64)
        oh[np.arange(T), idx[:, j]] = 1
        loc = np.cumsum(oh, axis=0) - 1 + counts[None, :]
        pos_k[:, j] = (loc * oh).sum(axis=1)
        counts = counts + oh.sum(axis=0)
        keep_k[:, j] = pos_k[:, j] < C

    kept = vals * keep_k
    w = kept / (kept.sum(axis=1, keepdims=True) + GATE_EPS)

    me = gates.mean(axis=0)
    ce = np.zeros((E,), np.float32)
    np.add.at(ce, idx[:, 0], 1.0)
    ce /= T
    l_aux = np.float32((me * ce).mean() * E * E)

    tok_for_slot = np.zeros((E, C), np.int64)
    for j in range(K):
        mk = keep_k[:, j]
        tok_for_slot[idx[mk, j], pos_k[mk, j]] = np.arange(T)[mk]

    return xn, idx, pos_k, keep_k, w, l_aux, counts.astype(np.float32), tok_for_slot


def _prep_in_maps(x, gamma, beta, wg, W1, b1, W2, b2):
    xn, idx, pos_k, keep_k, w, l_aux, counts, tok_for_slot = host_routing(
        np.asarray(x), np.asarray(gamma, np.float32), np.asarray(beta, np.float32),
        np.asarray(wg, np.float32))

    W1 = np.asarray(W1, np.float32)
    W2 = np.asarray(W2, np.float32)
    b1 = np.asarray(b1, np.float32)
    b2 = np.asarray(b2, np.float32)

    in_maps = []
    for e in range(E):
        xbuf = xn[tok_for_slot[e]]                       # [C, H]
        xt = np.ascontiguousarray(
            xbuf.T.reshape(KT1, P, C).transpose(1, 0, 2)).astype(np_bf16)
        w1r = np.ascontiguousarray(
            W1[e].reshape(KT1, P, MT1, P).transpose(1, 2, 0, 3)).astype(np_bf16)
        w2r = np.ascontiguousarray(
            W2[e].reshape(KT2, P, MT2, P).transpose(1, 2, 0, 3)).astype(np_bf16)
        b1r = np.ascontiguousarray(b1[e].reshape(MT1, P).T)
        b2r = np.ascontiguousarray(b2[e].reshape(MT2, P).T)
        in_maps.append({"xt": xt, "w1": w1r, "w2": w2r, "b1": b1r, "b2": b2r})

    routing = (xn, idx, pos_k, keep_k, w, l_aux, counts, tok_for_slot)
    return in_maps, routing


def _combine(x, results, routing):
    xn, idx, pos_k, keep_k, w, l_aux, counts, tok_for_slot = routing
    T = x.shape[0] * x.shape[1]
    # y[e] in [C, H] from yt [MT2, P, C]
    y = np.stack([
        results[e]["yt"].reshape(H, C).T for e in range(E)
    ])                                                   # [E, C, H]
    outf = np.zeros((T, H), np.float32)
    for j in range(TOP_K):
        mk = keep_k[:, j]
        outf[mk] += w[mk, j, None] * y[idx[mk, j], pos_k[mk, j]]
    out = np.asarray(x, np.float32) + outf.reshape(x.shape)
    return out, np.float32(l_aux), counts


def kernel(x, gamma, beta, wg, W1, b1, W2, b2):
    nc = _get_program(1)
    in_maps, routing = _prep_in_maps(x, gamma, beta, wg, W1, b1, W2, b2)
    res = run_bass_kernel_spmd(nc, in_maps, list(range(N_CORES)))
    return _combine(np.asarray(x), res.results, routing)


# revision 12
# speedup vs baseline: 940.8347x; 1.0456x over previous
"""DeepSpeed MoE block on 8 Trainium2 NeuronCores, expert-parallel.

Host: LayerNorm + gate + capacity-aware top-2 routing (cheap, O(T*E)) and the
dispatch/combine gathers. Device (per core = one expert): the dense expert MLP
y = gelu(xbuf @ W1 + b1) @ W2 + b2 over the expert's [C, H] token buffer —
>99.9% of the FLOPs — in bf16 with fp32 PSUM accumulation.

Self-contained: hardcodes shapes B=2, S=2048, H=1024, E=8, F=4096, K=2, C=1024.
"""

import numpy as np
import ml_dtypes

import concourse.bacc as bacc
import concourse.mybir as mybir
from concourse import tile
from concourse.bass_utils import run_bass_kernel_spmd

LN_EPS = 1e-5
GATE_EPS = 1e-9
TOP_K = 2
CAP_FACTOR = 1.0

N_CORES = 8
P = 128
H, F = 1024, 4096
E = 8
C = 1024  # int(ceil(TOP_K * 4096 / 8 * 1.0))

KT1 = H // P   # 8  k-tiles, MLP1 contraction
MT1 = F // P   # 32 m-tiles, MLP1 output partitions (F)
KT2 = F // P   # 32 k-tiles, MLP2 contraction
MT2 = H // P   # 8  m-tiles, MLP2 output partitions (H)
NCH = 512      # moving free dim per matmul
NN = C // NCH  # 2 column chunks

BF16 = mybir.dt.bfloat16
F32 = mybir.dt.float32
np_bf16 = np.dtype(ml_dtypes.bfloat16)


def _emit_body(nc, aps, sb, w1p, w2p, psp, outp, wconst=None,
               wsplit1=1, wsplit2=1):
    """One pass of the expert MLP (both matmuls + activations + output DMA)."""
    xt_sb, h1_sb, b1_sb, b2_sb = sb
    yt, w1, w2 = aps

    # MLP1: h1^T[F, C] = gelu(W1^T x^T + b1)
    for m in range(MT1):
        if wconst is not None:
            w1t = wconst[0]
        else:
            w1t = w1p.tile([P, KT1, P], BF16, name="w1t")
            for s_ in range(wsplit1):
                kk = KT1 // wsplit1
                nc.sync.dma_start(w1t[:, s_ * kk:(s_ + 1) * kk],
                                  w1[:, m, s_ * kk:(s_ + 1) * kk])
        ps = [psp.tile([P, NCH], F32, name=f"ps{n}", tag="ps") for n in range(NN)]
        for k in range(KT1):
            for n in range(NN):
                nc.tensor.matmul(
                    ps[n][:], w1t[:, k], xt_sb[:, k, n * NCH:(n + 1) * NCH],
                    start=(k == 0), stop=(k == KT1 - 1),
                )
        for n in range(NN):
            nc.scalar.activation(
                h1_sb[m][:, n * NCH:(n + 1) * NCH], ps[n][:],
                mybir.ActivationFunctionType.Gelu,
                bias=b1_sb[:, m:m + 1],
            )
    # MLP2: y^T[H, C] = W2^T h1^T + b2
    for m in range(MT2):
        if wconst is not None:
            w2t = wconst[1]
        else:
            w2t = w2p.tile([P, KT2, P], BF16, name="w2t")
            for s_ in range(wsplit2):
                kk = KT2 // wsplit2
                nc.sync.dma_start(w2t[:, s_ * kk:(s_ + 1) * kk],
                                  w2[:, m, s_ * kk:(s_ + 1) * kk])
        ps = [psp.tile([P, NCH], F32, name=f"ps{n}", tag="ps") for n in range(NN)]
        for k in range(KT2):
            for n in range(NN):
                nc.tensor.matmul(
                    ps[n][:], w2t[:, k], h1_sb[k][:, n * NCH:(n + 1) * NCH],
                    start=(k == 0), stop=(k == KT2 - 1),
                )
        o_sb = outp.tile([P, C], F32, name="o_sb")
        for n in range(NN):
            nc.vector.tensor_scalar_add(
                o_sb[:, n * NCH:(n + 1) * NCH], ps[n][:], b2_sb[:, m:m + 1]
            )
        nc.sync.dma_start(yt[m], o_sb[:])


def build_expert_program(repeats: int = 1, dyn_loop: bool = False,
                         w1_bufs: int = 5, w2_bufs: int = 4, ps_bufs: int = 6,
                         out_bufs: int = 4, skip_wdma: bool = False,
                         hint_engines=(), staggered_reset: bool = False,
                         wsplit1: int = 2, wsplit2: int = 4):
    """One expert's MLP: yT[m,p,c] = (gelu(x @ W1 + b1) @ W2 + b2)^T.

    Inputs are pre-laid-out host-side so every DMA is contiguous per partition:
      xt  [P, KT1, C]       bf16   xt[p,k,c]    = xbuf[c, k*P+p]
      w1  [P, MT1, KT1, P]  bf16   w1[p,m,k,q]  = W1[k*P+p, m*P+q]
      w2  [P, MT2, KT2, P]  bf16   w2[p,m,k,q]  = W2[k*P+p, m*P+q]
      b1  [P, MT1]          f32    b1[p,m]      = b1[m*P+p]
      b2  [P, MT2]          f32
    Output:
      yt  [MT2, P, C]       f32    yt[m,p,c]    = y[c, m*P+p]

    dyn_loop=True adds a `reps` int32 [1,1] input and wraps the body in a
    runtime For_i — used for timing (same NEFF, variable iteration count).
    """
    nc = bacc.Bacc("TRN2", target_bir_lowering=False, debug=False,
                   num_devices=N_CORES)
    xt = nc.dram_tensor("xt", [P, KT1, C], BF16, kind="ExternalInput").ap()
    w1 = nc.dram_tensor("w1", [P, MT1, KT1, P], BF16, kind="ExternalInput").ap()
    w2 = nc.dram_tensor("w2", [P, MT2, KT2, P], BF16, kind="ExternalInput").ap()
    b1 = nc.dram_tensor("b1", [P, MT1], F32, kind="ExternalInput").ap()
    b2 = nc.dram_tensor("b2", [P, MT2], F32, kind="ExternalInput").ap()
    if dyn_loop:
        reps = nc.dram_tensor("reps", [1, 1], mybir.dt.int32,
                              kind="ExternalInput").ap()
    yt = nc.dram_tensor("yt", [MT2, P, C], F32, kind="ExternalOutput").ap()

    with tile.TileContext(nc) as tc:
        with (
            tc.tile_pool(name="const", bufs=1) as const,
            tc.tile_pool(name="h1", bufs=1) as h1p,
            tc.tile_pool(name="w1p", bufs=w1_bufs) as w1p,
            tc.tile_pool(name="w2p", bufs=w2_bufs) as w2p,
            tc.tile_pool(name="psum", bufs=ps_bufs, space="PSUM") as psp,
            tc.tile_pool(name="out", bufs=out_bufs) as outp,
        ):
            xt_sb = const.tile([P, KT1, C], BF16)
            for k in range(KT1):
                nc.sync.dma_start(xt_sb[:, k], xt[:, k])
            b1_sb = const.tile([P, MT1], F32)
            nc.sync.dma_start(b1_sb[:], b1[:])
            b2_sb = const.tile([P, MT2], F32)
            nc.sync.dma_start(b2_sb[:], b2[:])

            h1_sb = [
                h1p.tile([P, C], BF16, tag=f"h1_{k}", name=f"h1_{k}")
                for k in range(KT2)
            ]

            sb = (xt_sb, h1_sb, b1_sb, b2_sb)
            aps = (yt, w1, w2)
            wconst = None
            if skip_wdma:
                w1c = const.tile([P, KT1, P], BF16)
                nc.sync.dma_start(w1c[:], w1[:, 0])
                w2c = const.tile([P, KT2, P], BF16)
                nc.sync.dma_start(w2c[:], w2[:, 0])
                wconst = (w1c, w2c)
            if dyn_loop:
                reps_sb = const.tile([1, 1], mybir.dt.int32)
                nc.sync.dma_start(reps_sb[:], reps[:])
                rv = nc.values_load(reps_sb[0:1, 0:1], min_val=0, max_val=1 << 20,
                                    skip_runtime_bounds_check=True)
                with tc.For_i(0, rv, 1, hint_engines=tuple(hint_engines),
                              staggered_reset=staggered_reset):
                    _emit_body(nc, aps, sb, w1p, w2p, psp, outp,
                               wconst=wconst, wsplit1=wsplit1, wsplit2=wsplit2)
            else:
                for _ in range(repeats):
                    _emit_body(nc, aps, sb, w1p, w2p, psp, outp,
                               wconst=wconst, wsplit1=wsplit1, wsplit2=wsplit2)

    nc.compile()
    return nc


_PROGRAM_CACHE = {}


def _get_program(repeats: int = 1):
    if repeats not in _PROGRAM_CACHE:
        _PROGRAM_CACHE[repeats] = build_expert_program(repeats)
    return _PROGRAM_CACHE[repeats]


def host_routing(x, gamma, beta, wg):
    """LayerNorm + softmax gate + capacity-aware top-2 routing (numpy fp32)."""
    B, S, _ = x.shape
    T = B * S
    K = TOP_K

    xf = x.reshape(T, H).astype(np.float32)
    mu = xf.mean(axis=-1, keepdims=True, dtype=np.float32)
    var = ((xf - mu) ** 2).mean(axis=-1, keepdims=True, dtype=np.float32)
    xn = (xf - mu) * (1.0 / np.sqrt(var + LN_EPS)) * gamma + beta

    logits = xn @ wg
    mx = logits.max(axis=-1, keepdims=True)
    eg = np.exp(logits - mx)
    gates = eg / eg.sum(axis=-1, keepdims=True)

    ordr = np.argsort(-gates, axis=1, kind="stable")[:, :K]
    idx = ordr.astype(np.int64)
    vals = np.take_along_axis(gates, ordr, axis=1)

    counts = np.zeros((E,), np.int64)
    pos_k = np.zeros((T, K), np.int64)
    keep_k = np.zeros((T, K), bool)
    for j in range(K):
        oh = np.zeros((T, E), np.int64)
        oh[np.arange(T), idx[:, j]] = 1
        loc = np.cumsum(oh, axis=0) - 1 + counts[None, :]
        pos_k[:, j] = (loc * oh).sum(axis=1)
        counts = counts + oh.sum(axis=0)
        keep_k[:, j] = pos_k[:, j] < C

    kept = vals * keep_k
    w = kept / (kept.sum(axis=1, keepdims=True) + GATE_EPS)

    me = gates.mean(axis=0)
    ce = np.zeros((E,), np.float32)
    np.add.at(ce, idx[:, 0], 1.0)
    ce /= T
    l_aux = np.float32((me * ce).mean() * E * E)

    tok_for_slot = np.zeros((E, C), np.int64)
    for j in range(K):
        mk = keep_k[:, j]
        tok_for_slot[idx[mk, j], pos_k[mk, j]] = np.arange(T)[mk]

    return xn, idx, pos_k, keep_k, w, l_aux, counts.astype(np.float32), tok_for_slot


def _prep_in_maps(x, gamma, beta, wg, W1, b1, W2, b2):
    xn, idx, pos_k, keep_k, w, l_aux, counts, tok_for_slot = host_routing(
        np.asarray(x), np.asarray(gamma, np.float32), np.asarray(beta, np.float32),
        np.asarray(wg, np.float32))

    W1 = np.asarray(W1, np.float32)
    W2 = np.asarray(W2, np.float32)
    b1 = np.asarray(b1, np.float32)
    b2 = np.asarray(b2, np.float32)

    in_maps = []
    for e in range(E):
        xbuf = xn[tok_for_slot[e]]                       # [C, H]
        xt = np.ascontiguousarray(
            xbuf.T.reshape(KT1, P, C).transpose(1, 0, 2)).astype(np_bf16)
        w1r = np.ascontiguousarray(
            W1[e].reshape(KT1, P, MT1, P).transpose(1, 2, 0, 3)).astype(np_bf16)
        w2r = np.ascontiguousarray(
            W2[e].reshape(KT2, P, MT2, P).transpose(1, 2, 0, 3)).astype(np_bf16)
        b1r = np.ascontiguousarray(b1[e].reshape(MT1, P).T)
        b2r = np.ascontiguousarray(b2[e].reshape(MT2, P).T)
        in_maps.append({"xt": xt, "w1": w1r, "w2": w2r, "b1": b1r, "b2": b2r})

    routing = (xn, idx, pos_k, keep_k, w, l_aux, counts, tok_for_slot)
    return in_maps, routing


def _combine(x, results, routing):
    xn, idx, pos_k, keep_k, w, l_aux, counts, tok_for_slot = routing
    T = x.shape[0] * x.shape[1]
    # y[e] in [C, H] from yt [MT2, P, C]
    y = np.stack([
        results[e]["yt"].reshape(H, C).T for e in range(E)
    ])                                                   # [E, C, H]
    outf = np.zeros((T, H), np.float32)
    for j in range(TOP_K):
        mk = keep_k[:, j]
        outf[mk] += w[mk, j, None] * y[idx[mk, j], pos_k[mk, j]]
    out = np.asarray(x, np.float32) + outf.reshape(x.shape)
    return out, np.float32(l_aux), counts


def kernel(x, gamma, beta, wg, W1, b1, W2, b2):
    nc = _get_program(1)
    in_maps, routing = _prep_in_maps(x, gamma, beta, wg, W1, b1, W2, b2)
    res = run_bass_kernel_spmd(nc, in_maps, list(range(N_CORES)))
    return _combine(np.asarray(x), res.results, routing)
